# revision 9
# baseline (speedup 1.0000x reference)
"""Trainium2 Bass kernel for nn_Attention_45148696216373.

8-core data-parallel over tokens (B*S = 131072 -> 16384/core), FEATURE-MAJOR
dataflow: x is host-transposed to [128c, tokens] fp16; every per-token tensor
lives as [feature-rows (partitions), token-columns (free)]. Per-token feature
reductions (attn sum over d, the k-sum, and the Wo projection) are then PE
matmuls with FIXED stationary matrices instead of Vector-engine work, and the
elementwise per-token products run as a handful of wide DVE/Pool ops.

Per 512-token tile (15 matmuls, rhs always 512 cols):
  proj (7 mm):  k0hat, mhat(=msum/4, k-dup), qhat_h (h=0..3, k-dup), vhat
                rows are [(k,d)] so downstream products align on partitions.
  km   (DVE):   custom fused op (k0hat + bk) * (mhat_staged + 0)   [1x, PSUM]
  P_h  (DVE):   custom fused op (qhat_h + bq_h) * km               [1x, PSUM]
  attn (4 mm):  attnrep_h = R^T @ P_h, R = (1/8) kronecker(I_k, ones_dd')
                -> attn[h,k] replicated over d on 128 rows.
  w'_h (Pool):  attnrep_staged * (vhat + bv)  [SBUF fp16, gpsimd]
  y_j  (4 mm):  y_j = Wt_0^T @ w'_{2j} + Wt_1^T @ w'_{2j+1},
                Wt_r[(k,d), o] = Wo[o, r*64+d]  (Wo applied on-chip)
  stage (Act):  mhat+bm, vhat+bv, attnrep copies (PSUM f32 -> SBUF fp16)
  drain (DVE):  y f32 -> fp16, DMA'd out feature-major; host transposes.

Biases ride free on the Act-Identity stages and the custom DVE ops'
per-partition scalars; scale folds: 1/H into mhat weights, 1/sqrt(D) into R.
Modeled ~150us/core vs 271us baseline (DVE-bound token-major).
"""

import os

if os.environ.get("JAX_PLATFORMS", "").strip().lower() == "cpu":
    os.environ.pop("JAX_PLATFORMS")

import numpy as np

B, S, DIM = 16, 8192, 128
H, KV, D = 4, 2, 64
T = B * S                 # 131072 tokens
NCORES = 8
TPC = T // NCORES         # 16384 tokens per core
TT = 512                  # tokens per tile
NT = TPC // TT            # 32 tiles per core

_COMPILED = None
_AFF_MUL = None


def _register_aff_mul():
    """Custom DVE op: out = (Src0 + C0) * (Src1 + C1); C0/C1 per-partition."""
    global _AFF_MUL
    if _AFF_MUL is not None:
        return _AFF_MUL
    import concourse.dve_ops as dvo
    from concourse.dve_spec import Spec, Src0, Src1, C0, C1, lower
    from concourse.dve_uop import DveOpSpec

    name = "AFF_MUL_ANT"
    if name in dvo._SUB_OPCODE_FOR_NAME:
        _AFF_MUL = next(op for op in dvo.OPS if op.name == name)
        return _AFF_MUL
    spec = Spec(
        body=(Src0 + C0) * (Src1 + C1),
        reference=lambda in0, in1, s0, s1: (in0 + s0) * (in1 + s1),
    )
    shas = {}
    for ver in ("v3", "v4"):
        tmp = DveOpSpec(name=name, opcode=0, uops=lower(spec, ver=ver),
                        rd1_en=True)
        shas[ver] = tmp.sha(ver)
    op = dvo.DveOp(name, spec, subdim=False, uops_sha=shas)
    dvo.OPS.append(op)
    dvo.CUSTOM_DVE_SPECS[name] = spec
    row = dvo._CUSTOM_DVE_ROW_BASE + len(dvo.OPS) - 1
    assert row < 0x20, row
    dvo._SUB_OPCODE_FOR_NAME[name] = row
    _AFF_MUL = op
    return op


def _fold_weights(Wq, bq, Wk, bk, Wv, bv, Wo):
    """Host-side weight prep. Returns (W_all [128,7,128] fp16,
    R [128,128] fp16, Wt [128,2,128] fp16, bias [128,7] f32).

    Feature rows are [(k,d)] = k*64+d. W_all chunk order:
    0: k0hat, 1: mhat (msum/4), 2..5: qhat_h (q[h,:] dup over k), 6: vhat.
    bias cols: 0 bk, 1 bm, 2 bv, 3..6 bq_h.
    """
    j = np.arange(H * D)
    Wq_p = Wq[j % H, j // H, :].astype(np.float64)        # [256, 128] rows q[h*64+d]
    bq_p = bq[j % H, j // H].astype(np.float64)           # [256]
    jk = np.arange(KV * D)
    Wk_p = Wk[jk % KV, jk // KV, :].astype(np.float64)    # [128, 128] rows k[k*64+d]
    bk_p = bk[jk % KV, jk // KV].astype(np.float64)
    Wv_p = Wv[jk % KV, jk // KV, :].astype(np.float64)
    bv_p = bv[jk % KV, jk // KV].astype(np.float64)

    Wm = Wq_p.reshape(H, D, DIM).sum(axis=0) / H          # [64, 128] msum/4
    bm = bq_p.reshape(H, D).sum(axis=0) / H               # [64]

    kk = jk // D   # k index of row (k,d)
    dd = jk % D    # d index

    W_all = np.zeros((128, 7, 128), dtype=np.float64)     # [c, chunk, (k,d)]
    bias = np.zeros((128, 7), dtype=np.float64)           # [(k,d), col]
    # chunk 0: k0hat
    W_all[:, 0, :] = Wk_p.T
    bias[:, 0] = bk_p
    # chunk 1: mhat (msum/4, dup over k)
    W_all[:, 1, :] = Wm[dd, :].T
    bias[:, 1] = bm[dd]
    # chunks 2..5: qhat_h = q[h, d] dup over k
    for h in range(H):
        W_all[:, 2 + h, :] = Wq_p[h * D + dd, :].T
        bias[:, 3 + h] = bq_p[h * D + dd]
    # chunk 6: vhat
    W_all[:, 6, :] = Wv_p.T
    bias[:, 2] = bv_p

    # R[(k,d), (k',d')] = (1/8) * (k==k')
    R = np.zeros((128, 128), dtype=np.float64)
    R[np.equal.outer(kk, kk)] = 1.0 / np.sqrt(D)
    # Wt_r[(k,d), o] = Wo[o, r*64+d]
    Wt = np.zeros((128, 2, 128), dtype=np.float64)
    for r in range(2):
        Wt[:, r, :] = Wo[:, r * D + dd].T

    return (W_all.astype(np.float16), R.astype(np.float16),
            Wt.astype(np.float16), bias.astype(np.float32))


def _build_program():
    import concourse.bass as bass
    import concourse.tile as tile
    from concourse import bacc, mybir

    aff_mul = _register_aff_mul()

    f32 = mybir.dt.float32
    fp16 = mybir.dt.float16
    ID = mybir.ActivationFunctionType.Identity

    nc = bacc.Bacc(
        "TRN2",
        target_bir_lowering=False,
        debug=False,
        enable_asserts=False,
        num_devices=NCORES,
    )

    xT_d = nc.dram_tensor("xT", [DIM, TPC], fp16, kind="ExternalInput").ap()
    w_d = nc.dram_tensor("wall", [DIM, 7, 128], fp16, kind="ExternalInput").ap()
    r_d = nc.dram_tensor("rmat", [128, 128], fp16, kind="ExternalInput").ap()
    wt_d = nc.dram_tensor("wtmat", [128, 2, 128], fp16, kind="ExternalInput").ap()
    b_d = nc.dram_tensor("bias", [128, 7], f32, kind="ExternalInput").ap()
    yT_d = nc.dram_tensor("yT", [2, DIM, TPC], fp16, kind="ExternalOutput").ap()

    with tile.TileContext(nc) as tc:
        with (
            tc.tile_pool(name="const", bufs=1) as cpool,
            tc.tile_pool(name="xin", bufs=3) as xpool,
            tc.tile_pool(name="psA", bufs=4, space="PSUM") as ppA,
            tc.tile_pool(name="psB", bufs=2, space="PSUM") as ppB,
            tc.tile_pool(name="stg", bufs=16) as spool,
            tc.tile_pool(name="prod", bufs=6) as qpool,
            tc.tile_pool(name="wout", bufs=6) as wpool,
            tc.tile_pool(name="yout", bufs=4) as ypool,
        ):
            w_sb = cpool.tile([DIM, 7, 128], fp16, name="w_sb")
            nc.sync.dma_start(w_sb[:], w_d[:, :, :])
            r_sb = cpool.tile([128, 128], fp16, name="r_sb")
            nc.sync.dma_start(r_sb[:], r_d[:, :])
            wt_sb = cpool.tile([128, 2, 128], fp16, name="wt_sb")
            nc.sync.dma_start(wt_sb[:], wt_d[:, :, :])
            b_sb = cpool.tile([128, 7], f32, name="b_sb")
            nc.sync.dma_start(b_sb[:], b_d[:, :])

            def head(i):
                """DMA-in, projection matmuls, km and P products for tile i."""
                xt = xpool.tile([DIM, TT], fp16, name="xt")
                nc.sync.dma_start(xt[:], xT_d[:, i * TT:(i + 1) * TT])

                pk = ppA.tile([128, TT], f32, name="pk", tag="ps")
                nc.tensor.matmul(out=pk[:], lhsT=w_sb[:, 0, :],
                                 rhs=xt[:], start=True, stop=True)
                pm = ppA.tile([128, TT], f32, name="pm", tag="ps")
                nc.tensor.matmul(out=pm[:], lhsT=w_sb[:, 1, :],
                                 rhs=xt[:], start=True, stop=True)
                pq = []
                for h in range(4):
                    t = ppA.tile([128, TT], f32, name=f"pq{h}", tag="ps")
                    nc.tensor.matmul(out=t[:], lhsT=w_sb[:, 2 + h, :],
                                     rhs=xt[:], start=True, stop=True)
                    pq.append(t)
                pv = ppA.tile([128, TT], f32, name="pv", tag="ps")
                nc.tensor.matmul(out=pv[:], lhsT=w_sb[:, 6, :],
                                 rhs=xt[:], start=True, stop=True)

                mb = spool.tile([128, TT], fp16, name="mb")
                nc.scalar.activation(mb[:], pm[:], ID,
                                     bias=b_sb[:, 1:2], scale=1.0)
                km = spool.tile([128, TT], fp16, name="km")
                nc.vector._custom_dve(aff_mul, out=km[:], in0=pk[:],
                                      in1=mb[:], s0=b_sb[:, 0:1], s1=0.0)
                vb = spool.tile([128, TT], fp16, name="vb")
                nc.scalar.activation(vb[:], pv[:], ID,
                                     bias=b_sb[:, 2:3], scale=1.0)

                P01 = qpool.tile([128, 2, TT], fp16, name="P01")
                P23 = qpool.tile([128, 2, TT], fp16, name="P23")
                for h in range(4):
                    dst = P01 if h < 2 else P23
                    nc.vector._custom_dve(aff_mul, out=dst[:, h % 2, :],
                                          in0=pq[h][:], in1=km[:],
                                          s0=b_sb[:, 3 + h:4 + h], s1=0.0)
                return P01, P23, vb

            def tail_a(i, P01, P23, vb):
                """attn matmuls, attn staging, w' products."""
                pa01 = ppB.tile([128, 2, TT], f32, name="pa01", tag="ps")
                nc.tensor.matmul(out=pa01[:, 0, :], lhsT=r_sb[:],
                                 rhs=P01[:, 0, :], start=True, stop=True)
                nc.tensor.matmul(out=pa01[:, 1, :], lhsT=r_sb[:],
                                 rhs=P01[:, 1, :], start=True, stop=True)
                ast0 = spool.tile([128, 2, TT], fp16, name="ast0")
                nc.scalar.copy(ast0[:], pa01[:])
                pa23 = ppB.tile([128, 2, TT], f32, name="pa23", tag="ps")
                nc.tensor.matmul(out=pa23[:, 0, :], lhsT=r_sb[:],
                                 rhs=P23[:, 0, :], start=True, stop=True)
                nc.tensor.matmul(out=pa23[:, 1, :], lhsT=r_sb[:],
                                 rhs=P23[:, 1, :], start=True, stop=True)
                ast1 = spool.tile([128, 2, TT], fp16, name="ast1")
                nc.scalar.copy(ast1[:], pa23[:])

                # w': 3 quarters on Pool, 1 quarter on DVE (2x, all-SBUF fp16)
                w0 = wpool.tile([128, 2, TT], fp16, name="w0")
                nc.gpsimd.tensor_mul(
                    w0[:], ast0[:],
                    vb[:].unsqueeze(1).broadcast_to([128, 2, TT]))
                w1 = wpool.tile([128, 2, TT], fp16, name="w1")
                nc.gpsimd.tensor_mul(w1[:, 0, :], ast1[:, 0, :], vb[:])
                nc.vector.tensor_mul(w1[:, 1, :], ast1[:, 1, :], vb[:])
                return w0, w1

            def tail_b(i, w0, w1):
                """y matmuls, drain, DMA-out."""
                pyt = ppB.tile([128, 2, TT], f32, name="pyt", tag="ps")
                nc.tensor.matmul(out=pyt[:, 0, :], lhsT=wt_sb[:, 0, :],
                                 rhs=w0[:, 0, :], start=True, stop=False)
                nc.tensor.matmul(out=pyt[:, 0, :], lhsT=wt_sb[:, 1, :],
                                 rhs=w0[:, 1, :], start=False, stop=True)
                nc.tensor.matmul(out=pyt[:, 1, :], lhsT=wt_sb[:, 0, :],
                                 rhs=w1[:, 0, :], start=True, stop=False)
                nc.tensor.matmul(out=pyt[:, 1, :], lhsT=wt_sb[:, 1, :],
                                 rhs=w1[:, 1, :], start=False, stop=True)

                yo = ypool.tile([128, 2, TT], fp16, name="yo")
                nc.vector.tensor_copy(yo[:, 0, :], pyt[:, 0, :])
                nc.scalar.copy(yo[:, 1, :], pyt[:, 1, :])
                for j in range(2):
                    nc.sync.dma_start(
                        yT_d[j, :, i * TT:(i + 1) * TT], yo[:, j, :]
                    )

            # software pipeline, 3-stage skew:
            #   head(i+1) | tail_a(i) | tail_b(i-1)
            hc = head(0)
            ac = None
            for i in range(NT + 1):
                if ac is not None:
                    tail_b(i - 1, *ac)
                nxt = head(i + 1) if i + 1 < NT else None
                na = tail_a(i, *hc) if i < NT else None
                hc, ac = nxt, na

    nc.compile()
    return nc


def kernel(x, Wq, bq, Wk, bk, Wv, bv, Wo):
    global _COMPILED
    from concourse.bass_utils import run_bass_kernel_spmd

    W_all, R, Wt, bias = _fold_weights(
        np.asarray(Wq, np.float64), np.asarray(bq, np.float64),
        np.asarray(Wk, np.float64), np.asarray(bk, np.float64),
        np.asarray(Wv, np.float64), np.asarray(bv, np.float64),
        np.asarray(Wo, np.float64),
    )

    if _COMPILED is None:
        _COMPILED = _build_program()
    nc = _COMPILED

    x2d = np.asarray(x, np.float32).reshape(T, DIM)
    in_maps = []
    for c in range(NCORES):
        shard = x2d[c * TPC:(c + 1) * TPC]
        in_maps.append({
            "xT": np.ascontiguousarray(shard.T).astype(np.float16),
            "wall": W_all,
            "rmat": R,
            "wtmat": Wt,
            "bias": bias,
        })

    res = run_bass_kernel_spmd(nc, in_maps, list(range(NCORES)))
    outs = []
    for c in range(NCORES):
        yT = res.results[c]["yT"]                     # [2, 128, TPC] fp16
        outs.append(np.transpose(yT, (2, 0, 1)))      # [TPC, 2, 128]
    Y = np.concatenate(outs, axis=0).astype(np.float32)   # [T, 2, 128]
    return np.ascontiguousarray(Y.reshape(B, 2 * S, DIM))


# revision 10
# speedup vs baseline: 1.1071x; 1.1071x over previous
"""Trainium2 Bass kernel for nn_Attention_45148696216373.

8-core data-parallel over tokens (B*S = 131072 -> 16384/core), FEATURE-MAJOR
dataflow: x is host-transposed to [128c, tokens] fp16; every per-token tensor
lives as [feature-rows (partitions), token-columns (free)]. Per-token feature
reductions (attn sum over d, the k-sum, and the Wo projection) are then PE
matmuls with FIXED stationary matrices instead of Vector-engine work, and the
elementwise per-token products run as a handful of wide DVE/Pool ops.

Per 512-token tile (15 matmuls, rhs always 512 cols):
  proj (7 mm):  k0hat, mhat(=msum/4, k-dup), qhat_h (h=0..3, k-dup), vhat
                rows are [(k,d)] so downstream products align on partitions.
  km   (DVE):   custom fused op (k0hat + bk) * (mhat_staged + 0)   [1x, PSUM]
  P_h  (DVE):   custom fused op (qhat_h + bq_h) * km               [1x, PSUM]
  attn (4 mm):  attnrep_h = R^T @ P_h, R = (1/8) kronecker(I_k, ones_dd')
                -> attn[h,k] replicated over d on 128 rows.
  w'_h (Pool):  attnrep_staged * (vhat + bv)  [SBUF fp16, gpsimd]
  y_j  (4 mm):  y_j = Wt_0^T @ w'_{2j} + Wt_1^T @ w'_{2j+1},
                Wt_r[(k,d), o] = Wo[o, r*64+d]  (Wo applied on-chip)
  stage (Act):  mhat+bm, vhat+bv, attnrep copies (PSUM f32 -> SBUF fp16)
  drain (DVE):  y f32 -> fp16, DMA'd out feature-major; host transposes.

Biases ride free on the Act-Identity stages and the custom DVE ops'
per-partition scalars; scale folds: 1/H into mhat weights, 1/sqrt(D) into R.
Modeled ~150us/core vs 271us baseline (DVE-bound token-major).
"""

import os

if os.environ.get("JAX_PLATFORMS", "").strip().lower() == "cpu":
    os.environ.pop("JAX_PLATFORMS")

import numpy as np

B, S, DIM = 16, 8192, 128
H, KV, D = 4, 2, 64
T = B * S                 # 131072 tokens
NCORES = 8
TPC = T // NCORES         # 16384 tokens per core
TT = 512                  # tokens per tile
NT = TPC // TT            # 32 tiles per core

_COMPILED = None
_AFF_MUL = None


def _register_aff_mul():
    """Custom DVE op: out = (Src0 + C0) * (Src1 + C1); C0/C1 per-partition."""
    global _AFF_MUL
    if _AFF_MUL is not None:
        return _AFF_MUL
    import concourse.dve_ops as dvo
    from concourse.dve_spec import Spec, Src0, Src1, C0, C1, lower
    from concourse.dve_uop import DveOpSpec

    name = "AFF_MUL_ANT"
    if name in dvo._SUB_OPCODE_FOR_NAME:
        _AFF_MUL = next(op for op in dvo.OPS if op.name == name)
        return _AFF_MUL
    spec = Spec(
        body=(Src0 + C0) * (Src1 + C1),
        reference=lambda in0, in1, s0, s1: (in0 + s0) * (in1 + s1),
    )
    shas = {}
    for ver in ("v3", "v4"):
        tmp = DveOpSpec(name=name, opcode=0, uops=lower(spec, ver=ver),
                        rd1_en=True)
        shas[ver] = tmp.sha(ver)
    op = dvo.DveOp(name, spec, subdim=False, uops_sha=shas)
    dvo.OPS.append(op)
    dvo.CUSTOM_DVE_SPECS[name] = spec
    row = dvo._CUSTOM_DVE_ROW_BASE + len(dvo.OPS) - 1
    assert row < 0x20, row
    dvo._SUB_OPCODE_FOR_NAME[name] = row
    _AFF_MUL = op
    return op


def _fold_weights(Wq, bq, Wk, bk, Wv, bv, Wo):
    """Host-side weight prep. Returns (W_all [128,7,128] fp16,
    R [128,128] fp16, Wt [128,2,128] fp16, bias [128,7] f32).

    Feature rows are [(k,d)] = k*64+d. W_all chunk order:
    0: k0hat, 1: mhat (msum/4), 2..5: qhat_h (q[h,:] dup over k), 6: vhat.
    bias cols: 0 bk, 1 bm, 2 bv, 3..6 bq_h.
    """
    j = np.arange(H * D)
    Wq_p = Wq[j % H, j // H, :].astype(np.float64)        # [256, 128] rows q[h*64+d]
    bq_p = bq[j % H, j // H].astype(np.float64)           # [256]
    jk = np.arange(KV * D)
    Wk_p = Wk[jk % KV, jk // KV, :].astype(np.float64)    # [128, 128] rows k[k*64+d]
    bk_p = bk[jk % KV, jk // KV].astype(np.float64)
    Wv_p = Wv[jk % KV, jk // KV, :].astype(np.float64)
    bv_p = bv[jk % KV, jk // KV].astype(np.float64)

    Wm = Wq_p.reshape(H, D, DIM).sum(axis=0) / H          # [64, 128] msum/4
    bm = bq_p.reshape(H, D).sum(axis=0) / H               # [64]

    kk = jk // D   # k index of row (k,d)
    dd = jk % D    # d index

    W_all = np.zeros((128, 7, 128), dtype=np.float64)     # [c, chunk, (k,d)]
    bias = np.zeros((128, 7), dtype=np.float64)           # [(k,d), col]
    # chunk 0: k0hat
    W_all[:, 0, :] = Wk_p.T
    bias[:, 0] = bk_p
    # chunk 1: mhat (msum/4, dup over k)
    W_all[:, 1, :] = Wm[dd, :].T
    bias[:, 1] = bm[dd]
    # chunks 2..5: qhat_h = q[h, d] dup over k
    for h in range(H):
        W_all[:, 2 + h, :] = Wq_p[h * D + dd, :].T
        bias[:, 3 + h] = bq_p[h * D + dd]
    # chunk 6: vhat
    W_all[:, 6, :] = Wv_p.T
    bias[:, 2] = bv_p

    # R[(k,d), (k',d')] = (1/8) * (k==k')
    R = np.zeros((128, 128), dtype=np.float64)
    R[np.equal.outer(kk, kk)] = 1.0 / np.sqrt(D)
    # Wt_r[(k,d), o] = Wo[o, r*64+d]
    Wt = np.zeros((128, 2, 128), dtype=np.float64)
    for r in range(2):
        Wt[:, r, :] = Wo[:, r * D + dd].T

    return (W_all.astype(np.float16), R.astype(np.float16),
            Wt.astype(np.float16), bias.astype(np.float32))


def _build_program():
    import concourse.bass as bass
    import concourse.tile as tile
    from concourse import bacc, mybir

    aff_mul = _register_aff_mul()

    f32 = mybir.dt.float32
    fp16 = mybir.dt.float16
    ID = mybir.ActivationFunctionType.Identity

    nc = bacc.Bacc(
        "TRN2",
        target_bir_lowering=False,
        debug=False,
        enable_asserts=False,
        num_devices=NCORES,
    )

    xT_d = nc.dram_tensor("xT", [DIM, TPC], fp16, kind="ExternalInput").ap()
    w_d = nc.dram_tensor("wall", [DIM, 7, 128], fp16, kind="ExternalInput").ap()
    r_d = nc.dram_tensor("rmat", [128, 128], fp16, kind="ExternalInput").ap()
    wt_d = nc.dram_tensor("wtmat", [128, 2, 128], fp16, kind="ExternalInput").ap()
    b_d = nc.dram_tensor("bias", [128, 7], f32, kind="ExternalInput").ap()
    yT_d = nc.dram_tensor("yT", [2, DIM, TPC], fp16, kind="ExternalOutput").ap()

    with tile.TileContext(nc) as tc:
        with (
            tc.tile_pool(name="const", bufs=1) as cpool,
            tc.tile_pool(name="xin", bufs=3) as xpool,
            tc.tile_pool(name="psA", bufs=4, space="PSUM") as ppA,
            tc.tile_pool(name="psB", bufs=2, space="PSUM") as ppB,
            tc.tile_pool(name="stg", bufs=16) as spool,
            tc.tile_pool(name="prod", bufs=6) as qpool,
            tc.tile_pool(name="wout", bufs=6) as wpool,
            tc.tile_pool(name="yout", bufs=4) as ypool,
        ):
            w_sb = cpool.tile([DIM, 7, 128], fp16, name="w_sb")
            nc.sync.dma_start(w_sb[:], w_d[:, :, :])
            r_sb = cpool.tile([128, 128], fp16, name="r_sb")
            nc.sync.dma_start(r_sb[:], r_d[:, :])
            wt_sb = cpool.tile([128, 2, 128], fp16, name="wt_sb")
            nc.sync.dma_start(wt_sb[:], wt_d[:, :, :])
            b_sb = cpool.tile([128, 7], f32, name="b_sb")
            nc.sync.dma_start(b_sb[:], b_d[:, :])

            def head(i):
                """DMA-in, projection matmuls, km and P products for tile i."""
                xt = xpool.tile([DIM, TT], fp16, name="xt")
                nc.sync.dma_start(xt[:], xT_d[:, i * TT:(i + 1) * TT])

                pk = ppA.tile([128, TT], f32, name="pk", tag="ps")
                nc.tensor.matmul(out=pk[:], lhsT=w_sb[:, 0, :],
                                 rhs=xt[:], start=True, stop=True)
                pm = ppA.tile([128, TT], f32, name="pm", tag="ps")
                nc.tensor.matmul(out=pm[:], lhsT=w_sb[:, 1, :],
                                 rhs=xt[:], start=True, stop=True)
                pq = []
                for h in range(4):
                    t = ppA.tile([128, TT], f32, name=f"pq{h}", tag="ps")
                    nc.tensor.matmul(out=t[:], lhsT=w_sb[:, 2 + h, :],
                                     rhs=xt[:], start=True, stop=True)
                    pq.append(t)
                pv = ppA.tile([128, TT], f32, name="pv", tag="ps")
                nc.tensor.matmul(out=pv[:], lhsT=w_sb[:, 6, :],
                                 rhs=xt[:], start=True, stop=True)

                mb = spool.tile([128, TT], fp16, name="mb")
                nc.scalar.activation(mb[:], pm[:], ID,
                                     bias=b_sb[:, 1:2], scale=1.0)
                km = spool.tile([128, TT], fp16, name="km")
                nc.vector._custom_dve(aff_mul, out=km[:], in0=pk[:],
                                      in1=mb[:], s0=b_sb[:, 0:1], s1=0.0)
                vb = spool.tile([128, TT], fp16, name="vb")
                nc.scalar.activation(vb[:], pv[:], ID,
                                     bias=b_sb[:, 2:3], scale=1.0)

                P01 = qpool.tile([128, 2, TT], fp16, name="P01")
                P23 = qpool.tile([128, 2, TT], fp16, name="P23")
                for h in range(4):
                    dst = P01 if h < 2 else P23
                    nc.vector._custom_dve(aff_mul, out=dst[:, h % 2, :],
                                          in0=pq[h][:], in1=km[:],
                                          s0=b_sb[:, 3 + h:4 + h], s1=0.0)
                return P01, P23, vb

            def tail_a(i, P01, P23, vb):
                """attn matmuls, attn staging, w' products."""
                pa01 = ppB.tile([128, 2, TT], f32, name="pa01", tag="ps")
                nc.tensor.matmul(out=pa01[:, 0, :], lhsT=r_sb[:],
                                 rhs=P01[:, 0, :], start=True, stop=True)
                nc.tensor.matmul(out=pa01[:, 1, :], lhsT=r_sb[:],
                                 rhs=P01[:, 1, :], start=True, stop=True)
                ast0 = spool.tile([128, 2, TT], fp16, name="ast0")
                nc.scalar.copy(ast0[:], pa01[:])
                pa23 = ppB.tile([128, 2, TT], f32, name="pa23", tag="ps")
                nc.tensor.matmul(out=pa23[:, 0, :], lhsT=r_sb[:],
                                 rhs=P23[:, 0, :], start=True, stop=True)
                nc.tensor.matmul(out=pa23[:, 1, :], lhsT=r_sb[:],
                                 rhs=P23[:, 1, :], start=True, stop=True)
                ast1 = spool.tile([128, 2, TT], fp16, name="ast1")
                nc.scalar.copy(ast1[:], pa23[:])

                # w': 3 quarters on Pool, 1 quarter on DVE (2x, all-SBUF fp16)
                w0 = wpool.tile([128, 2, TT], fp16, name="w0")
                nc.gpsimd.tensor_mul(
                    w0[:], ast0[:],
                    vb[:].unsqueeze(1).broadcast_to([128, 2, TT]))
                w1 = wpool.tile([128, 2, TT], fp16, name="w1")
                nc.gpsimd.tensor_mul(w1[:, 0, :], ast1[:, 0, :], vb[:])
                nc.vector.tensor_mul(w1[:, 1, :], ast1[:, 1, :], vb[:])
                return w0, w1

            def tail_b(i, w0, w1):
                """y matmuls, drain, DMA-out."""
                pyt = ppB.tile([128, 2, TT], f32, name="pyt", tag="ps")
                nc.tensor.matmul(out=pyt[:, 0, :], lhsT=wt_sb[:, 0, :],
                                 rhs=w0[:, 0, :], start=True, stop=False)
                nc.tensor.matmul(out=pyt[:, 0, :], lhsT=wt_sb[:, 1, :],
                                 rhs=w0[:, 1, :], start=False, stop=True)
                nc.tensor.matmul(out=pyt[:, 1, :], lhsT=wt_sb[:, 0, :],
                                 rhs=w1[:, 0, :], start=True, stop=False)
                nc.tensor.matmul(out=pyt[:, 1, :], lhsT=wt_sb[:, 1, :],
                                 rhs=w1[:, 1, :], start=False, stop=True)

                yo = ypool.tile([128, 2, TT], fp16, name="yo")
                nc.vector.tensor_copy(yo[:, 0, :], pyt[:, 0, :])
                nc.scalar.copy(yo[:, 1, :], pyt[:, 1, :])
                for j in range(2):
                    nc.sync.dma_start(
                        yT_d[j, :, i * TT:(i + 1) * TT], yo[:, j, :]
                    )

            # software pipeline, 3-stage skew:
            #   head(i+1) | tail_a(i) | tail_b(i-1)
            hc = head(0)
            ac = None
            for i in range(NT + 1):
                nxt = head(i + 1) if i + 1 < NT else None
                na = tail_a(i, *hc) if i < NT else None
                if ac is not None:
                    tail_b(i - 1, *ac)
                hc, ac = nxt, na

    nc.compile()
    return nc


def kernel(x, Wq, bq, Wk, bk, Wv, bv, Wo):
    global _COMPILED
    from concourse.bass_utils import run_bass_kernel_spmd

    W_all, R, Wt, bias = _fold_weights(
        np.asarray(Wq, np.float64), np.asarray(bq, np.float64),
        np.asarray(Wk, np.float64), np.asarray(bk, np.float64),
        np.asarray(Wv, np.float64), np.asarray(bv, np.float64),
        np.asarray(Wo, np.float64),
    )

    if _COMPILED is None:
        _COMPILED = _build_program()
    nc = _COMPILED

    x2d = np.asarray(x, np.float32).reshape(T, DIM)
    in_maps = []
    for c in range(NCORES):
        shard = x2d[c * TPC:(c + 1) * TPC]
        in_maps.append({
            "xT": np.ascontiguousarray(shard.T).astype(np.float16),
            "wall": W_all,
            "rmat": R,
            "wtmat": Wt,
            "bias": bias,
        })

    res = run_bass_kernel_spmd(nc, in_maps, list(range(NCORES)))
    outs = []
    for c in range(NCORES):
        yT = res.results[c]["yT"]                     # [2, 128, TPC] fp16
        outs.append(np.transpose(yT, (2, 0, 1)))      # [TPC, 2, 128]
    Y = np.concatenate(outs, axis=0).astype(np.float32)   # [T, 2, 128]
    return np.ascontiguousarray(Y.reshape(B, 2 * S, DIM))


# revision 11
# speedup vs baseline: 1.3040x; 1.1778x over previous
"""Trainium2 Bass kernel for nn_Attention_45148696216373.

8-core data-parallel over tokens (B*S = 131072 -> 16384/core), FEATURE-MAJOR
dataflow: x is host-transposed to [128c, tokens] fp16; every per-token tensor
lives as [feature-rows (partitions), token-columns (free)]. Per-token feature
reductions (attn sum over d, the k-sum, and the Wo projection) are then PE
matmuls with FIXED stationary matrices instead of Vector-engine work, and the
elementwise per-token products run as a handful of wide DVE/Pool ops.

Per 512-token tile (15 matmuls, rhs always 512 cols):
  proj (7 mm):  k0hat, mhat(=msum/4, k-dup), qhat_h (h=0..3, k-dup), vhat
                rows are [(k,d)] so downstream products align on partitions.
  km   (DVE):   custom fused op (k0hat + bk) * (mhat_staged + 0)   [1x, PSUM]
  P_h  (DVE):   custom fused op (qhat_h + bq_h) * km               [1x, PSUM]
  attn (4 mm):  attnrep_h = R^T @ P_h, R = (1/8) kronecker(I_k, ones_dd')
                -> attn[h,k] replicated over d on 128 rows.
  w'_h (Pool):  attnrep_staged * (vhat + bv)  [SBUF fp16, gpsimd]
  y_j  (4 mm):  y_j = Wt_0^T @ w'_{2j} + Wt_1^T @ w'_{2j+1},
                Wt_r[(k,d), o] = Wo[o, r*64+d]  (Wo applied on-chip)
  stage (Act):  mhat+bm, vhat+bv, attnrep copies (PSUM f32 -> SBUF fp16)
  drain (DVE):  y f32 -> fp16, DMA'd out feature-major; host transposes.

Biases ride free on the Act-Identity stages and the custom DVE ops'
per-partition scalars; scale folds: 1/H into mhat weights, 1/sqrt(D) into R.
Modeled ~150us/core vs 271us baseline (DVE-bound token-major).
"""

import os

if os.environ.get("JAX_PLATFORMS", "").strip().lower() == "cpu":
    os.environ.pop("JAX_PLATFORMS")

import numpy as np

B, S, DIM = 16, 8192, 128
H, KV, D = 4, 2, 64
T = B * S                 # 131072 tokens
NCORES = 8
TPC = T // NCORES         # 16384 tokens per core
TT = 512                  # tokens per tile
NT = TPC // TT            # 32 tiles per core

_COMPILED = None
_AFF_MUL = None


def _register_aff_mul():
    """Custom DVE op: out = (Src0 + C0) * (Src1 + C1); C0/C1 per-partition."""
    global _AFF_MUL
    if _AFF_MUL is not None:
        return _AFF_MUL
    import concourse.dve_ops as dvo
    from concourse.dve_spec import Spec, Src0, Src1, C0, C1, lower
    from concourse.dve_uop import DveOpSpec

    name = "AFF_MUL_ANT"
    if name in dvo._SUB_OPCODE_FOR_NAME:
        _AFF_MUL = next(op for op in dvo.OPS if op.name == name)
        return _AFF_MUL
    spec = Spec(
        body=(Src0 + C0) * (Src1 + C1),
        reference=lambda in0, in1, s0, s1: (in0 + s0) * (in1 + s1),
    )
    shas = {}
    for ver in ("v3", "v4"):
        tmp = DveOpSpec(name=name, opcode=0, uops=lower(spec, ver=ver),
                        rd1_en=True)
        shas[ver] = tmp.sha(ver)
    op = dvo.DveOp(name, spec, subdim=False, uops_sha=shas)
    dvo.OPS.append(op)
    dvo.CUSTOM_DVE_SPECS[name] = spec
    row = dvo._CUSTOM_DVE_ROW_BASE + len(dvo.OPS) - 1
    assert row < 0x20, row
    dvo._SUB_OPCODE_FOR_NAME[name] = row
    _AFF_MUL = op
    return op


def _fold_weights(Wq, bq, Wk, bk, Wv, bv, Wo):
    """Host-side weight prep. Returns (W_all [128,7,128] fp16,
    R [128,128] fp16, Wt [128,2,128] fp16, bias [128,7] f32).

    Feature rows are [(k,d)] = k*64+d. W_all chunk order:
    0: k0hat, 1: mhat (msum/4), 2..5: qhat_h (q[h,:] dup over k), 6: vhat.
    bias cols: 0 bk, 1 bm, 2 bv, 3..6 bq_h.
    """
    j = np.arange(H * D)
    Wq_p = Wq[j % H, j // H, :].astype(np.float64)        # [256, 128] rows q[h*64+d]
    bq_p = bq[j % H, j // H].astype(np.float64)           # [256]
    jk = np.arange(KV * D)
    Wk_p = Wk[jk % KV, jk // KV, :].astype(np.float64)    # [128, 128] rows k[k*64+d]
    bk_p = bk[jk % KV, jk // KV].astype(np.float64)
    Wv_p = Wv[jk % KV, jk // KV, :].astype(np.float64)
    bv_p = bv[jk % KV, jk // KV].astype(np.float64)

    Wm = Wq_p.reshape(H, D, DIM).sum(axis=0) / H          # [64, 128] msum/4
    bm = bq_p.reshape(H, D).sum(axis=0) / H               # [64]

    kk = jk // D   # k index of row (k,d)
    dd = jk % D    # d index

    W_all = np.zeros((128, 7, 128), dtype=np.float64)     # [c, chunk, (k,d)]
    bias = np.zeros((128, 7), dtype=np.float64)           # [(k,d), col]
    # chunk 0: k0hat
    W_all[:, 0, :] = Wk_p.T
    bias[:, 0] = bk_p
    # chunk 1: mhat (msum/4, dup over k)
    W_all[:, 1, :] = Wm[dd, :].T
    bias[:, 1] = bm[dd]
    # chunks 2..5: qhat_h = q[h, d] dup over k
    for h in range(H):
        W_all[:, 2 + h, :] = Wq_p[h * D + dd, :].T
        bias[:, 3 + h] = bq_p[h * D + dd]
    # chunk 6: vhat
    W_all[:, 6, :] = Wv_p.T
    bias[:, 2] = bv_p

    # R[(k,d), (k',d')] = (1/8) * (k==k')
    R = np.zeros((128, 128), dtype=np.float64)
    R[np.equal.outer(kk, kk)] = 1.0 / np.sqrt(D)
    # Wt_r[(k,d), o] = Wo[o, r*64+d]
    Wt = np.zeros((128, 2, 128), dtype=np.float64)
    for r in range(2):
        Wt[:, r, :] = Wo[:, r * D + dd].T

    return (W_all.astype(np.float16), R.astype(np.float16),
            Wt.astype(np.float16), bias.astype(np.float32))


def _build_program():
    import concourse.bass as bass
    import concourse.tile as tile
    from concourse import bacc, mybir

    aff_mul = _register_aff_mul()

    f32 = mybir.dt.float32
    fp16 = mybir.dt.float16
    ID = mybir.ActivationFunctionType.Identity

    nc = bacc.Bacc(
        "TRN2",
        target_bir_lowering=False,
        debug=False,
        enable_asserts=False,
        num_devices=NCORES,
    )

    xT_d = nc.dram_tensor("xT", [DIM, TPC], fp16, kind="ExternalInput").ap()
    w_d = nc.dram_tensor("wall", [DIM, 7, 128], fp16, kind="ExternalInput").ap()
    r_d = nc.dram_tensor("rmat", [128, 128], fp16, kind="ExternalInput").ap()
    wt_d = nc.dram_tensor("wtmat", [128, 2, 128], fp16, kind="ExternalInput").ap()
    b_d = nc.dram_tensor("bias", [128, 7], f32, kind="ExternalInput").ap()
    yT_d = nc.dram_tensor("yT", [2, DIM, TPC], fp16, kind="ExternalOutput").ap()

    with tile.TileContext(nc) as tc:
        with (
            tc.tile_pool(name="const", bufs=1) as cpool,
            tc.tile_pool(name="xin", bufs=3) as xpool,
            tc.tile_pool(name="psA", bufs=4, space="PSUM") as ppA,
            tc.tile_pool(name="psB", bufs=2, space="PSUM") as ppB,
            tc.tile_pool(name="stg", bufs=16) as spool,
            tc.tile_pool(name="prod", bufs=6) as qpool,
            tc.tile_pool(name="wout", bufs=6) as wpool,
            tc.tile_pool(name="yout", bufs=4) as ypool,
        ):
            w_sb = cpool.tile([DIM, 7, 128], fp16, name="w_sb")
            nc.sync.dma_start(w_sb[:], w_d[:, :, :])
            r_sb = cpool.tile([128, 128], fp16, name="r_sb")
            nc.sync.dma_start(r_sb[:], r_d[:, :])
            wt_sb = cpool.tile([128, 2, 128], fp16, name="wt_sb")
            nc.sync.dma_start(wt_sb[:], wt_d[:, :, :])
            b_sb = cpool.tile([128, 7], f32, name="b_sb")
            nc.sync.dma_start(b_sb[:], b_d[:, :])

            def head1(i):
                """DMA-in, k/m projection, mb stage, km product for tile i."""
                xt = xpool.tile([DIM, TT], fp16, name="xt")
                nc.sync.dma_start(xt[:], xT_d[:, i * TT:(i + 1) * TT])

                pk = ppA.tile([128, TT], f32, name="pk", tag="ps")
                nc.tensor.matmul(out=pk[:], lhsT=w_sb[:, 0, :],
                                 rhs=xt[:], start=True, stop=True)
                pm = ppA.tile([128, TT], f32, name="pm", tag="ps")
                nc.tensor.matmul(out=pm[:], lhsT=w_sb[:, 1, :],
                                 rhs=xt[:], start=True, stop=True)
                mb = spool.tile([128, TT], fp16, name="mb")
                nc.scalar.activation(mb[:], pm[:], ID,
                                     bias=b_sb[:, 1:2], scale=1.0)
                km = spool.tile([128, TT], fp16, name="km")
                nc.vector._custom_dve(aff_mul, out=km[:], in0=pk[:],
                                      in1=mb[:], s0=b_sb[:, 0:1], s1=0.0)
                return xt, km

            def head2(i, xt, km):
                """q/v projections, vb stage, P products for tile i."""
                pq = []
                for h in range(4):
                    t = ppA.tile([128, TT], f32, name=f"pq{h}", tag="ps")
                    nc.tensor.matmul(out=t[:], lhsT=w_sb[:, 2 + h, :],
                                     rhs=xt[:], start=True, stop=True)
                    pq.append(t)
                pv = ppA.tile([128, TT], f32, name="pv", tag="ps")
                nc.tensor.matmul(out=pv[:], lhsT=w_sb[:, 6, :],
                                 rhs=xt[:], start=True, stop=True)
                vb = spool.tile([128, TT], fp16, name="vb")
                nc.scalar.activation(vb[:], pv[:], ID,
                                     bias=b_sb[:, 2:3], scale=1.0)

                P01 = qpool.tile([128, 2, TT], fp16, name="P01")
                P23 = qpool.tile([128, 2, TT], fp16, name="P23")
                for h in range(4):
                    dst = P01 if h < 2 else P23
                    nc.vector._custom_dve(aff_mul, out=dst[:, h % 2, :],
                                          in0=pq[h][:], in1=km[:],
                                          s0=b_sb[:, 3 + h:4 + h], s1=0.0)
                return P01, P23, vb

            def tail_a(i, P01, P23, vb):
                """attn matmuls, attn staging, w' products."""
                pa01 = ppB.tile([128, 2, TT], f32, name="pa01", tag="ps")
                nc.tensor.matmul(out=pa01[:, 0, :], lhsT=r_sb[:],
                                 rhs=P01[:, 0, :], start=True, stop=True)
                nc.tensor.matmul(out=pa01[:, 1, :], lhsT=r_sb[:],
                                 rhs=P01[:, 1, :], start=True, stop=True)
                ast0 = spool.tile([128, 2, TT], fp16, name="ast0")
                nc.scalar.copy(ast0[:], pa01[:])
                pa23 = ppB.tile([128, 2, TT], f32, name="pa23", tag="ps")
                nc.tensor.matmul(out=pa23[:, 0, :], lhsT=r_sb[:],
                                 rhs=P23[:, 0, :], start=True, stop=True)
                nc.tensor.matmul(out=pa23[:, 1, :], lhsT=r_sb[:],
                                 rhs=P23[:, 1, :], start=True, stop=True)
                ast1 = spool.tile([128, 2, TT], fp16, name="ast1")
                nc.scalar.copy(ast1[:], pa23[:])

                # w': 3 quarters on Pool, 1 quarter on DVE (2x, all-SBUF fp16)
                w0 = wpool.tile([128, 2, TT], fp16, name="w0")
                nc.gpsimd.tensor_mul(
                    w0[:], ast0[:],
                    vb[:].unsqueeze(1).broadcast_to([128, 2, TT]))
                w1 = wpool.tile([128, 2, TT], fp16, name="w1")
                nc.gpsimd.tensor_mul(w1[:, 0, :], ast1[:, 0, :], vb[:])
                nc.vector.tensor_mul(w1[:, 1, :], ast1[:, 1, :], vb[:])
                return w0, w1

            def tail_b(i, w0, w1):
                """y matmuls, drain, DMA-out."""
                pyt = ppB.tile([128, 2, TT], f32, name="pyt", tag="ps")
                nc.tensor.matmul(out=pyt[:, 0, :], lhsT=wt_sb[:, 0, :],
                                 rhs=w0[:, 0, :], start=True, stop=False)
                nc.tensor.matmul(out=pyt[:, 0, :], lhsT=wt_sb[:, 1, :],
                                 rhs=w0[:, 1, :], start=False, stop=True)
                nc.tensor.matmul(out=pyt[:, 1, :], lhsT=wt_sb[:, 0, :],
                                 rhs=w1[:, 0, :], start=True, stop=False)
                nc.tensor.matmul(out=pyt[:, 1, :], lhsT=wt_sb[:, 1, :],
                                 rhs=w1[:, 1, :], start=False, stop=True)

                yo = ypool.tile([128, 2, TT], fp16, name="yo")
                nc.vector.tensor_copy(yo[:, 0, :], pyt[:, 0, :])
                nc.scalar.copy(yo[:, 1, :], pyt[:, 1, :])
                for j in range(2):
                    nc.sync.dma_start(
                        yT_d[j, :, i * TT:(i + 1) * TT], yo[:, j, :]
                    )

            # software pipeline, 4-stage skew:
            #   head1(i+2) | head2(i+1) | tail_a(i) | tail_b(i-1)
            h1c = head1(0)
            h2c = None
            ac = None
            for i in range(-1, NT + 1):
                n1 = head1(i + 2) if 0 <= i + 2 < NT else None
                n2 = head2(i + 1, *h1c) if i + 1 < NT and h1c is not None else None
                na = tail_a(i, *h2c) if 0 <= i < NT and h2c is not None else None
                if ac is not None:
                    tail_b(i - 1, *ac)
                h1c, h2c, ac = n1, n2, na

    nc.compile()
    return nc


def kernel(x, Wq, bq, Wk, bk, Wv, bv, Wo):
    global _COMPILED
    from concourse.bass_utils import run_bass_kernel_spmd

    W_all, R, Wt, bias = _fold_weights(
        np.asarray(Wq, np.float64), np.asarray(bq, np.float64),
        np.asarray(Wk, np.float64), np.asarray(bk, np.float64),
        np.asarray(Wv, np.float64), np.asarray(bv, np.float64),
        np.asarray(Wo, np.float64),
    )

    if _COMPILED is None:
        _COMPILED = _build_program()
    nc = _COMPILED

    x2d = np.asarray(x, np.float32).reshape(T, DIM)
    in_maps = []
    for c in range(NCORES):
        shard = x2d[c * TPC:(c + 1) * TPC]
        in_maps.append({
            "xT": np.ascontiguousarray(shard.T).astype(np.float16),
            "wall": W_all,
            "rmat": R,
            "wtmat": Wt,
            "bias": bias,
        })

    res = run_bass_kernel_spmd(nc, in_maps, list(range(NCORES)))
    outs = []
    for c in range(NCORES):
        yT = res.results[c]["yT"]                     # [2, 128, TPC] fp16
        outs.append(np.transpose(yT, (2, 0, 1)))      # [TPC, 2, 128]
    Y = np.concatenate(outs, axis=0).astype(np.float32)   # [T, 2, 128]
    return np.ascontiguousarray(Y.reshape(B, 2 * S, DIM))
